# revision 1
# baseline (speedup 1.0000x reference)
"""Distributed real-vector SHT on 8 Trainium2 NeuronCores.

Full inputs in, full output out. Internally: azimuthal-mode (m) model
parallelism — each of the 8 cores computes 46 of the 368 (padded from 361)
azimuthal modes end-to-end:

  stage 1 (DFT):      y[m, r] = sum_n dft[n, m] * x[n, r]      (matmul)
  stage 2 (transp):   y_T[lat, (comp,trig,ch), m]              (PE transpose)
  stage 3 (Legendre): out[(comp,trig,ch), l] = sum_lat y_T * w (matmul per m)
  stage 4 (combine):  complex recombination of the 4 planes    (DVE adds)

All tensor math happens on-device; the host only does layout shuffles,
dtype casts and the final complex packing.
"""

import sys
import numpy as np
from contextlib import ExitStack

sys.path.insert(0, "/opt/trn_rl_repo")

import concourse.bass as bass  # noqa: E402
import concourse.tile as tile  # noqa: E402
from concourse import bacc  # noqa: E402
from concourse import mybir  # noqa: E402
from concourse.bass_utils import run_bass_kernel_spmd  # noqa: E402

NLAT, NLON = 360, 720
LMAX, MMAX = 360, 361
NCORES = 8
MPC = 46           # modes per core (8*46 = 368 >= 361, padded with zeros)
M2 = 2 * MPC       # 92: cos block + sin block
PADM = 128         # DFT output partitions: cos at 0:46, sin at 64:110
KC = 120           # contraction chunk (partitions)
NKC = 6            # longitude chunks: 6*120 = 720
LKC = 3            # latitude chunks:  3*120 = 360
CH = 32
R = 2 * CH * NLAT  # 23040 rows = (comp, ch, lat)
RT = 720           # row tile = 2 (comp,ch) pairs of one component
NRT = R // RT      # 32
NPR = RT // NLAT   # 2 pairs per row tile
F16 = mybir.dt.float16
F32 = mybir.dt.float32

_CACHE = {}


def _build_program(reps=1, mode="full"):
    nc = bacc.Bacc("TRN2", target_bir_lowering=False, debug=False,
                   num_devices=NCORES)
    xt = nc.dram_tensor("xt", [KC, NKC, R], F16, kind="ExternalInput").ap()
    dftm = nc.dram_tensor("dftm", [KC, NKC, PADM], F16, kind="ExternalInput").ap()
    gmat = nc.dram_tensor("gmat", [PADM, 2, PADM], F16, kind="ExternalInput").ap()
    wts = nc.dram_tensor("wts", [MPC, KC, LKC, 2 * LMAX], F16,
                         kind="ExternalInput").ap()
    out = nc.dram_tensor("out", [MPC, 64, 2, LMAX], F16,
                         kind="ExternalOutput").ap()

    with tile.TileContext(nc) as tc, ExitStack() as ctx:
        const_pool = ctx.enter_context(tc.tile_pool(name="const", bufs=1))
        yt_pool = ctx.enter_context(tc.tile_pool(name="yt", bufs=1))

        df_t = const_pool.tile([KC, NKC, PADM], F16, tag="df")
        nc.gpsimd.dma_start(df_t[:], dftm)
        g_t = const_pool.tile([PADM, 2, PADM], F16, tag="g")
        nc.gpsimd.dma_start(g_t[:], gmat)

        # y_T[kc]: [lat_part, j, (comp, trig, ch)]
        yt_t = [yt_pool.tile([KC, MPC, 2, 2, CH], F16, tag=f"yt{kc}",
                             name=f"yt{kc}")
                for kc in range(LKC)]

        if mode == "nodve":
            for kc in range(LKC):
                nc.gpsimd.memset(yt_t[kc][:], 0.0)
        for _rep in range(reps):
            _build_body(nc, tc, xt, dftm, gmat, wts, out, df_t, g_t, yt_t, mode)

    nc.compile()
    return nc


def _build_body(nc, tc, xt, dftm, gmat, wts, out, df_t, g_t, yt_t, mode="full"):
    dma_only = (mode == "dma")
    no_dve = (mode in ("dma", "nodve"))
    if True:
        # ---- stage 1+2: DFT + transpose ----
        with tc.tile_pool(name="xin", bufs=3) as xin_pool, \
             tc.tile_pool(name="dps", bufs=2, space="PSUM") as dps_pool, \
             tc.tile_pool(name="ycp", bufs=4) as yc_pool, \
             tc.tile_pool(name="tps", bufs=4, space="PSUM") as tps_pool:
            for t in range(NRT):
                comp = t // (NRT // 2)
                x_t = xin_pool.tile([KC, NKC, RT], F16, tag="xin")
                nc.gpsimd.dma_start(x_t[:], xt[:, :, t * RT:(t + 1) * RT])

                ps = dps_pool.tile([PADM, 2, 512], F32, tag="dps")
                for h in range(2 if not dma_only else 0):
                    for kc in range(NKC):
                        nc.tensor.matmul(
                            ps[:, h, 0:360],
                            lhsT=df_t[:, kc, :],
                            rhs=x_t[:, kc, h * 360:(h + 1) * 360],
                            start=(kc == 0), stop=(kc == NKC - 1),
                        )

                yc = yc_pool.tile([PADM, RT], F16, tag="yc")
                ycv = yc.rearrange("p (a b) -> p a b", a=2, b=360)
                psv = ps[:, :, 0:360]
                if no_dve:
                    if not dma_only:
                        nc.gpsimd.memset(yc[:], 0.0)
                elif comp == 0:
                    # negate sin rows so block values are (y0r, y0i);
                    # split cos half onto ACT to offload DVE
                    nc.scalar.copy(ycv[0:64], psv[0:64])
                    nc.vector.tensor_scalar_mul(ycv[64:PADM],
                                                psv[64:PADM], -1.0)
                else:
                    nc.scalar.copy(ycv[0:64], psv[0:64])
                    nc.vector.tensor_copy(ycv[64:PADM], psv[64:PADM])

                for c in range(NPR if not dma_only else 0):
                    ch = (t % (NRT // 2)) * NPR + c
                    for kc in range(LKC):
                        tp = tps_pool.tile([KC, 2, 64], F16, tag="tps")
                        nc.tensor.transpose(
                            tp[:],
                            yc[:, c * NLAT + kc * KC:c * NLAT + (kc + 1) * KC],
                            g_t[:, comp, :],
                        )
                        if not no_dve:
                            # scatter -> yt[kc][:, j, comp, trig, ch]
                            dst = yt_t[kc][:, :, comp, :, ch].transpose([0, 2, 1])
                            if kc == 1:
                                nc.scalar.copy(dst, tp[:, :, 0:MPC])
                            else:
                                nc.vector.tensor_copy(dst, tp[:, :, 0:MPC])

        # ---- stage 3+4: Legendre + combine ----
        with tc.tile_pool(name="win", bufs=20) as w_pool, \
             tc.tile_pool(name="lps", bufs=4, space="PSUM") as lps_pool, \
             tc.tile_pool(name="osb", bufs=8) as o_pool:
            for j in range(MPC):
                w_t = w_pool.tile([KC, LKC, 2 * LMAX], F16, tag="win")
                nc.gpsimd.dma_start(w_t[:], wts[j])

                lp = lps_pool.tile([128, 2, 512], F32, tag="lps")
                for h in range(2 if not dma_only else 0):
                    for kc in range(LKC):
                        nc.tensor.matmul(
                            lp[:, h, 0:LMAX],
                            lhsT=yt_t[kc][:, j],
                            rhs=w_t[:, kc, h * LMAX:(h + 1) * LMAX],
                            start=(kc == 0), stop=(kc == LKC - 1),
                        )

                osb = o_pool.tile([64, 2, LMAX], F16, tag="osb")
                if not no_dve:
                    # DVE TensorTensor may read only one PSUM operand: stage
                    # the comp-1 half through SBUF on the idle ScalarE.
                    csb = o_pool.tile([64, 2, LMAX], F32, tag="csb")
                    nc.scalar.copy(csb[:], lp[64:128, :, 0:LMAX])
                    # rows: [out0re; out0im]
                    nc.vector.tensor_add(osb[:, 0, :], lp[0:64, 0, 0:LMAX],
                                         csb[:, 1, :])
                    # rows: [out1im; -out1re]; sign of out1re fixed on host
                    nc.vector.tensor_add(osb[:, 1, :], lp[0:64, 1, 0:LMAX],
                                         csb[:, 0, :])
                else:
                    nc.gpsimd.memset(osb[:], 0.0)

                nc.gpsimd.dma_start(out[j], osb[:])


def _prep_in_maps(x, weights):
    x = np.asarray(x, dtype=np.float32)
    weights = np.asarray(weights, dtype=np.float32)

    # xt[p, kc, r]: longitude-on-partitions view of x, r = (comp, ch, lat)
    xf = np.transpose(x[0], (3, 1, 0, 2)).reshape(NLON, R)
    xt = np.ascontiguousarray(
        xf.reshape(NKC, KC, R).transpose(1, 0, 2)).astype(np.float16)

    # permutation matrices for the PE transposes (must be square 0/1 perms)
    g = np.zeros((PADM, 2, PADM), dtype=np.float16)
    g[:, 0, :] = np.eye(PADM, dtype=np.float16)      # comp0: identity
    for i in range(64):
        g[64 + i, 1, i] = 1    # comp1: sin block -> slot block 0
        g[i, 1, 64 + i] = 1    # comp1: cos block -> slot block 1

    n = np.arange(NLON, dtype=np.float64)
    in_maps = []
    for c in range(NCORES):
        mb = c * MPC
        m = mb + np.arange(MPC, dtype=np.float64)
        ang = 2.0 * np.pi * np.outer(n, m) / NLON
        s = 2.0 * np.pi / NLON
        cosm = np.cos(ang) * s
        sinm = np.sin(ang) * s
        valid = (mb + np.arange(MPC)) < MMAX
        cosm[:, ~valid] = 0.0
        sinm[:, ~valid] = 0.0
        dft = np.zeros((NLON, PADM), dtype=np.float64)
        dft[:, 0:MPC] = cosm
        dft[:, 64:64 + MPC] = sinm
        dftm = np.ascontiguousarray(
            dft.reshape(NKC, KC, PADM).transpose(1, 0, 2)).astype(np.float16)

        take = max(0, min(MPC, MMAX - mb))
        wc = np.zeros((2, MPC, LMAX, NLAT), dtype=np.float32)
        if take:
            wc[:, :take] = weights[:, mb:mb + take]
        # wts[j, p, kc, i*360 + l] = wc[i, j, l, kc*120 + p]
        tmp = wc.transpose(1, 3, 0, 2)                      # (j, k, i, l)
        tmp = tmp.reshape(MPC, LKC, KC, 2, LMAX)
        tmp = tmp.transpose(0, 2, 1, 3, 4)                  # (j, p, kc, i, l)
        wts = np.ascontiguousarray(
            tmp.reshape(MPC, KC, LKC, 2 * LMAX)).astype(np.float16)

        in_maps.append({"xt": xt, "dftm": dftm, "gmat": g, "wts": wts})
    return in_maps


def _assemble(results):
    full = np.empty((1, CH, 2, LMAX, MMAX), dtype=np.complex64)
    for c in range(NCORES):
        mb = c * MPC
        take = max(0, min(MPC, MMAX - mb))
        if not take:
            continue
        o = results[c]["out"].astype(np.float32)  # [46, 64, 2, 360]
        out0 = (o[:, 0:CH, 0, :] + 1j * o[:, CH:64, 0, :]).astype(np.complex64)
        out1 = (-o[:, CH:64, 1, :] + 1j * o[:, 0:CH, 1, :]).astype(np.complex64)
        # (j, ch, l) -> (ch, l, j)
        full[0, :, 0, :, mb:mb + take] = out0.transpose(1, 2, 0)[:, :, :take]
        full[0, :, 1, :, mb:mb + take] = out1.transpose(1, 2, 0)[:, :, :take]
    return full


def _run(x, weights, trace=False):
    if "nc" not in _CACHE:
        _CACHE["nc"] = _build_program()
    nc = _CACHE["nc"]
    in_maps = _prep_in_maps(x, weights)
    res = run_bass_kernel_spmd(nc, in_maps, list(range(NCORES)), trace=trace)
    return _assemble(res.results), res


def kernel(x, weights):
    out, _ = _run(x, weights, trace=False)
    return out



# revision 2
# speedup vs baseline: 1.7273x; 1.7273x over previous
"""Distributed real-vector SHT on 8 Trainium2 NeuronCores.

Full inputs in, full output out. Internally: parity-split azimuthal-mode
model parallelism. Cores 0-3 take the 181 even m (46+45+45+45), cores 4-7
the 180 odd m (45 each). A mode of parity p only needs the folded input
e/o(n) = x(n) +/- x(n+360) (n < 360), so each core DMAs HALF of x and the
DFT contraction is 360-long instead of 720.

  stage 1 (DFT):  psum[lat, m, trig] += eo[lon, lat]^T @ trig[lon, m]
                  (x-tile is the STATIONARY operand, so the output lands
                  lat-major — no PE transposes needed)
  scatter:        psum -> ytA = [C0, -S0, -C1, S1], ytB = [S1, C1, S0, C0]
                  (plane arrangements with signs folded in)
  stage 2 (Leg):  out[(grp,ch), l] = sum_lat ytA*w0 + ytB*w1   (PSUM
                  accumulation performs the complex recombination for free)

out rows (grp,ch): grp0=Re(out0), grp1=Im(out0), grp2=Re(out1), grp3=Im(out1).
All tensor math happens on-device; the host does the parity fold, layout
shuffles, dtype casts and the final complex packing.
"""

import sys
import numpy as np
from contextlib import ExitStack

sys.path.insert(0, "/opt/trn_rl_repo")

import concourse.bass as bass  # noqa: E402
import concourse.tile as tile  # noqa: E402
from concourse import bacc  # noqa: E402
from concourse import mybir  # noqa: E402
from concourse.bass_utils import run_bass_kernel_spmd  # noqa: E402

NLAT, NLON = 360, 720
LMAX, MMAX = 360, 361
NCORES = 8
MPC = 46           # modes per core (padded; even: 46/45/45/45, odd: 45 x4)
CH = 32
KC = 120           # partition chunk
F16 = mybir.dt.float16
F32 = mybir.dt.float32

_CACHE = {}


def _build_program(reps=1, mode="full"):
    nc = bacc.Bacc("TRN2", target_bir_lowering=False, debug=False,
                   num_devices=NCORES)
    xt = nc.dram_tensor("xt", [KC, 3, 64, 360], F16, kind="ExternalInput").ap()
    dftm = nc.dram_tensor("dftm", [KC, 3, MPC, 2], F16,
                          kind="ExternalInput").ap()
    wts = nc.dram_tensor("wts", [MPC, KC, 3, 2 * LMAX], F16,
                         kind="ExternalInput").ap()
    out = nc.dram_tensor("out", [128, MPC, LMAX], F16,
                         kind="ExternalOutput").ap()

    with tile.TileContext(nc) as tc, ExitStack() as ctx:
        const_pool = ctx.enter_context(tc.tile_pool(name="const", bufs=1))
        yt_pool = ctx.enter_context(tc.tile_pool(name="yt", bufs=1))

        df_t = const_pool.tile([KC, 3, MPC, 2], F16, tag="df")
        nc.gpsimd.dma_start(df_t[:], dftm)

        # ytA/ytB: [lat-in-chunk, lat-chunk, j, grp4, ch32]
        ytA = yt_pool.tile([KC, 3, MPC, 4, CH], F16, tag="ytA", name="ytA")
        ytB = yt_pool.tile([KC, 3, MPC, 4, CH], F16, tag="ytB", name="ytB")

        if mode in ("dma", "nodve"):
            nc.gpsimd.memset(ytA[:], 0.0)
            nc.gpsimd.memset(ytB[:], 0.0)
        for _rep in range(reps):
            _build_body(nc, tc, xt, wts, out, df_t, ytA, ytB, mode)

    nc.compile()
    return nc


def _build_body(nc, tc, xt, wts, out, df_t, ytA, ytB, mode="full"):
    dma_only = (mode == "dma")
    no_dve = (mode in ("dma", "nodve"))

    # ---- stage 1: DFT (+ scatter into ytA/ytB) ----
    with tc.tile_pool(name="xin", bufs=3) as xin_pool, \
         tc.tile_pool(name="dps", bufs=2, space="PSUM") as dps_pool:
        for g in range(16):             # 4-channel pair groups
            comp = g // 8
            ch0 = (g % 8) * 4
            x_t = xin_pool.tile([KC, 3, 4, 360], F16, tag="xin")
            nc.gpsimd.dma_start(x_t[:], xt[:, :, 4 * g:4 * g + 4, :])

            # [lat, lat-chunk(lb), ci, m(pad64), trig]
            ps = dps_pool.tile([KC, 3, 4, 64, 2], F32, tag="dps")
            for lb in range(3 if not dma_only else 0):
                for ci in range(4):
                    for kc in range(3):
                        nc.tensor.matmul(
                            ps[:, lb, ci, 0:MPC, :],
                            lhsT=x_t[:, kc, ci, lb * KC:(lb + 1) * KC],
                            rhs=df_t[:, kc],
                            start=(kc == 0), stop=(kc == 2),
                        )
            if no_dve:
                continue
            # psum -> yt scatter (signs folded); src [120, lb, j, ci]
            srcC = ps[:, :, :, 0:MPC, 0].transpose([0, 1, 3, 2])
            srcS = ps[:, :, :, 0:MPC, 1].transpose([0, 1, 3, 2])
            if comp == 0:
                # A: [C0, -S0, ...]   B: [..., S0, C0]
                nc.vector.tensor_scalar_mul(
                    ytA[:, :, :, 1, ch0:ch0 + 4], srcS, -1.0)
                nc.vector.tensor_copy(ytB[:, :, :, 2, ch0:ch0 + 4], srcS)
                nc.scalar.copy(ytA[:, :, :, 0, ch0:ch0 + 4], srcC)
                nc.scalar.copy(ytB[:, :, :, 3, ch0:ch0 + 4], srcC)
            else:
                # A: [..., -C1, S1]   B: [S1, C1, ...]
                nc.vector.tensor_scalar_mul(
                    ytA[:, :, :, 2, ch0:ch0 + 4], srcC, -1.0)
                nc.vector.tensor_copy(ytB[:, :, :, 1, ch0:ch0 + 4], srcC)
                nc.scalar.copy(ytA[:, :, :, 3, ch0:ch0 + 4], srcS)
                nc.scalar.copy(ytB[:, :, :, 0, ch0:ch0 + 4], srcS)

    # ---- stage 2: Legendre + combine-in-PSUM ----
    with tc.tile_pool(name="win", bufs=16) as w_pool, \
         tc.tile_pool(name="lps", bufs=4, space="PSUM") as lps_pool, \
         tc.tile_pool(name="osb", bufs=3) as o_pool:
        osb = None
        for j in range(MPC):
            w_t = w_pool.tile([KC, 3, 2 * LMAX], F16, tag="win")
            nc.gpsimd.dma_start(w_t[:], wts[j])

            lp = lps_pool.tile([128, 512], F32, tag="lps")
            for lb in range(3 if not dma_only else 0):
                nc.tensor.matmul(
                    lp[:, 0:LMAX], lhsT=ytA[:, lb, j], rhs=w_t[:, lb, 0:LMAX],
                    start=(lb == 0), stop=False,
                )
            for lb in range(3 if not dma_only else 0):
                nc.tensor.matmul(
                    lp[:, 0:LMAX], lhsT=ytB[:, lb, j],
                    rhs=w_t[:, lb, LMAX:2 * LMAX],
                    start=False, stop=(lb == 2),
                )

            if j % 4 == 0:
                osb = o_pool.tile([128, 4, LMAX], F16, tag="osb")
            if no_dve:
                if j % 4 == 0 and dma_only:
                    nc.gpsimd.memset(osb[:], 0.0)
            elif j % 2 == 0:
                nc.vector.tensor_copy(osb[:, j % 4, :], lp[:, 0:LMAX])
            else:
                nc.scalar.copy(osb[:, j % 4, :], lp[:, 0:LMAX])
            if j % 4 == 3 or j == MPC - 1:
                jb = (j // 4) * 4
                nc.gpsimd.dma_start(out[:, jb:j + 1, :],
                                    osb[:, 0:j + 1 - jb, :])


def _mode_sets():
    even_m = np.arange(0, MMAX, 2)
    odd_m = np.arange(1, MMAX, 2)
    sets, o = [], 0
    for cnt in (46, 45, 45, 45):
        sets.append(even_m[o:o + cnt]); o += cnt
    o = 0
    for cnt in (45, 45, 45, 45):
        sets.append(odd_m[o:o + cnt]); o += cnt
    return sets


def _prep_in_maps(x, weights):
    x = np.asarray(x, dtype=np.float32)
    weights = np.asarray(weights, dtype=np.float32)

    xf = x[0]                                  # (ch, comp, lat, lon)
    e = xf[..., :360] + xf[..., 360:]
    o = xf[..., :360] - xf[..., 360:]

    def pack_x(src):
        # -> xt[p, kc, comp*32+ch, lat], lon = kc*120 + p
        a = src.transpose(3, 1, 0, 2)          # (lon, comp, ch, lat)
        a = a.reshape(3, KC, 2, CH, 360).transpose(1, 0, 2, 3, 4)
        return np.ascontiguousarray(
            a.reshape(KC, 3, 64, 360)).astype(np.float16)

    xt_eo = [pack_x(e), pack_x(o)]

    s = 2.0 * np.pi / NLON
    n = np.arange(360, dtype=np.float64)
    in_maps = []
    for c, ms in enumerate(_mode_sets()):
        nm = len(ms)
        ang = 2.0 * np.pi * np.outer(n, ms.astype(np.float64)) / NLON
        dft = np.zeros((360, MPC, 2), dtype=np.float64)
        dft[:, :nm, 0] = np.cos(ang) * s
        dft[:, :nm, 1] = np.sin(ang) * s
        dftm = np.ascontiguousarray(
            dft.reshape(3, KC, MPC, 2).transpose(1, 0, 2, 3)
        ).astype(np.float16)

        wc = np.zeros((2, MPC, LMAX, NLAT), dtype=np.float32)
        wc[:, :nm] = weights[:, ms]
        tmp = wc.transpose(1, 3, 0, 2)                  # (j, lat, i, l)
        tmp = tmp.reshape(MPC, 3, KC, 2, LMAX).transpose(0, 2, 1, 3, 4)
        w_c = np.ascontiguousarray(
            tmp.reshape(MPC, KC, 3, 2 * LMAX)).astype(np.float16)

        in_maps.append({"xt": xt_eo[c // 4], "dftm": dftm, "wts": w_c})
    return in_maps


def _assemble(results):
    full = np.empty((1, CH, 2, LMAX, MMAX), dtype=np.complex64)
    for c, ms in enumerate(_mode_sets()):
        nm = len(ms)
        o = results[c]["out"].astype(np.float32)   # (128, 46, 360)
        g = o.reshape(4, CH, MPC, LMAX)
        out0 = (g[0] + 1j * g[1]).astype(np.complex64)   # (ch, j, l)
        out1 = (g[2] + 1j * g[3]).astype(np.complex64)
        full[0, :, 0, :, ms] = out0[:, :nm].transpose(1, 0, 2)
        full[0, :, 1, :, ms] = out1[:, :nm].transpose(1, 0, 2)
    return full


def _run(x, weights, trace=False):
    if "nc" not in _CACHE:
        _CACHE["nc"] = _build_program()
    nc = _CACHE["nc"]
    in_maps = _prep_in_maps(x, weights)
    res = run_bass_kernel_spmd(nc, in_maps, list(range(NCORES)), trace=trace)
    return _assemble(res.results), res


def kernel(x, weights):
    out, _ = _run(x, weights, trace=False)
    return out
